# revision 20
# baseline (speedup 1.0000x reference)
"""LoRALinear kernel for Trainium2 (8 NeuronCores, SPMD data-parallel).

Computes out = x @ W.T + b + SCALE*((x@gA.T)@gB.T + (x@lA.T)@lB.T)
  x: [8, 2048, 1024] f32, W: [4096, 1024], b: [4096]
  gA/lA: [8, 1024], gB/lB: [4096, 8]  ->  out: [8, 2048, 4096] f32

Strategy (per core, one batch of x each):
  1. Merge LoRA into W_effT[k, o] = W.T + (SCALE*A_cat).T @ B_catT on
     device: W cast to fp16 on ScalarE, PE-transposed 4 blocks per
     [128,512] fp16 PSUM tile, rank-16 fp16 LoRA matmul evicted by
     ScalarE, summed into fp16 SBUF with one DVE op per tile.
  2. Main matmul per 128-row s-tile: cast x to fp16 (ScalarE),
     PE-transpose blocks to x.T (grouped 4-per-PSUM-tile, one DVE
     eviction each), accumulate psum[s,o] over 8 k-tiles of fp16
     matmuls (all-fp16 keeps LDWEIGHTS pipelined -> ~N cycles/matmul);
     bias added in f32 on DVE during psum eviction.

fp16 operand rounding gives ~3e-4 absmax relative error vs the f32
reference (validated numerically); accumulation stays f32 in PSUM.

Host only shards x over cores, stacks/pre-scales the rank-8 adapters
(A_cat = SCALE*[gA;lA], B_catT = [gB.T;lB.T]) and casts b to fp16 -
O(r*d) marshaling.
"""
import numpy as np
from contextlib import ExitStack

import concourse.bass as bass
import concourse.tile as tile
from concourse import bacc, mybir
from concourse.bass import ts, ds
from concourse.bass_utils import run_bass_kernel_spmd
from concourse.masks import make_identity

F32 = mybir.dt.float32
F16 = mybir.dt.float16

N_CORES = 8
B, S, DIN, DOUT, R = 8, 2048, 1024, 4096, 8
SCALE = 16.0 / 8
R2 = 2 * R

P = 128            # partition tile
OTILE = 512        # matmul moving free dim (one PSUM bank of f32)
KT = DIN // P      # 8 k-tiles
OT = DOUT // OTILE # 8 o-tiles
ST = S // P        # 16 s-tiles


def build_nc():
    nc = bacc.Bacc("TRN2", target_bir_lowering=False, debug=False,
                   num_devices=N_CORES)
    x = nc.dram_tensor("x", [S, DIN], F32, kind="ExternalInput").ap()
    W = nc.dram_tensor("W", [DOUT, DIN], F32, kind="ExternalInput").ap()
    bvec = nc.dram_tensor("b16", [DOUT], F16, kind="ExternalInput").ap()
    A_cat = nc.dram_tensor("A_cat", [R2, DIN], F16, kind="ExternalInput").ap()
    B_catT = nc.dram_tensor("B_catT", [R2, DOUT], F16, kind="ExternalInput").ap()
    out = nc.dram_tensor("out", [S, DOUT], F32, kind="ExternalOutput").ap()

    with tile.TileContext(nc) as tc:
        with ExitStack() as ctx:
            const = ctx.enter_context(tc.tile_pool(name="const", bufs=1))
            wet_pool = ctx.enter_context(tc.tile_pool(name="wet", bufs=1))
            wslab_pool = ctx.enter_context(tc.tile_pool(name="wslab", bufs=3))
            xin_pool = ctx.enter_context(tc.tile_pool(name="xin", bufs=2))
            xt_pool = ctx.enter_context(tc.tile_pool(name="xt", bufs=2))
            out_pool = ctx.enter_context(tc.tile_pool(name="outp", bufs=4))
            psh = ctx.enter_context(tc.tile_pool(name="psh", bufs=2, space="PSUM"))
            ps512 = ctx.enter_context(tc.tile_pool(name="ps512", bufs=6, space="PSUM"))

            # ---- constants ----
            ident_h = const.tile([P, P], F16)
            make_identity(nc, ident_h)

            acat = const.tile([R2, DIN], F16)
            nc.sync.dma_start(acat[:], A_cat)
            bcatt = const.tile([R2, DOUT], F16)
            nc.sync.dma_start(bcatt[:], B_catT)

            # bias broadcast to all 128 partitions via rank-1 fp16 matmul
            ones_col = const.tile([1, P], F16)
            nc.vector.memset(ones_col[:], 1.0)
            brow16 = const.tile([1, DOUT], F16)
            nc.sync.dma_start(brow16[:], bvec[None, :])
            bias_sb = const.tile([P, DOUT], F32)
            for ot in range(OT):
                pb = ps512.tile([P, OTILE], F32, tag="ps512")
                nc.tensor.matmul(pb[:], ones_col[:],
                                 brow16[:, ts(ot, OTILE)],
                                 start=True, stop=True)
                nc.vector.tensor_copy(bias_sb[:, ts(ot, OTILE)], pb[:])

            # ---- phase 1: W_effT[k%128, k//128, o] in SBUF, fp16 ----
            # W.T lands via XBAR transpose-DMAs (Activation HWDGE); the
            # rank-16 LoRA matmul is then added in place on DVE.
            wet = wet_pool.tile([P, KT, DOUT], F16)
            for ot in range(OT):
                for j in range(4):
                    ob = 4 * ot + j
                    wsl = wslab_pool.tile([P, DIN], F32, tag="wslab32")
                    nc.sync.dma_start(wsl[:], W[ts(ob, P), :])
                    w16 = wslab_pool.tile([P, DIN], F16, tag=f"wslab16_{j}",
                                          name=f"w16_{ot}_{j}")
                    nc.scalar.copy(w16[:], wsl[:])
                    # XBAR: wet[k2, k1, ob*128 + o] = w16[o, k]
                    nc.scalar.dma_start_transpose(wet[:, :, ts(ob, P)], w16[:])
                for kt in range(KT):
                    # LoRA term: (SCALE*A_cat).T @ B_catT chunk, K=16
                    pl = ps512.tile([P, OTILE], F32, tag="ps512")
                    nc.tensor.matmul(pl[:], acat[:, ts(kt, P)],
                                     bcatt[:, ts(ot, OTILE)],
                                     start=True, stop=True)
                    wchunk = wet[:, kt, ts(ot, OTILE)]
                    nc.vector.tensor_tensor(wchunk, pl[:], wchunk,
                                            mybir.AluOpType.add)

            # ---- phase 2: out[s, o] = x @ W_effT + bias ----
            for st in range(ST):
                xin = xin_pool.tile([P, DIN], F32)
                nc.sync.dma_start(xin[:], x[ts(st, P), :])
                x16 = xin_pool.tile([P, DIN], F16, tag="x16")
                nc.scalar.copy(x16[:], xin[:])
                xt = xt_pool.tile([P, KT * P], F16)
                for g in range(KT // 4):
                    pxq = psh.tile([P, OTILE], F16, tag="psh")
                    for j in range(4):
                        nc.tensor.matmul(pxq[:, ts(j, P)],
                                         x16[:, ts(4 * g + j, P)], ident_h[:],
                                         is_transpose=True,
                                         start=(j == 0), stop=(j == 3))
                    nc.vector.tensor_copy(xt[:, ts(g, OTILE)], pxq[:])
                for ot in range(OT):
                    po = ps512.tile([P, OTILE], F32, tag="ps512")
                    for kt in range(KT):
                        nc.tensor.matmul(po[:], xt[:, ts(kt, P)],
                                         wet[:, kt, ts(ot, OTILE)],
                                         start=(kt == 0), stop=(kt == KT - 1))
                    osb = out_pool.tile([P, OTILE], F32)
                    nc.vector.tensor_tensor(osb[:], po[:],
                                            bias_sb[:, ts(ot, OTILE)],
                                            mybir.AluOpType.add)
                    nc.sync.dma_start(out[ts(st, P), ts(ot, OTILE)], osb[:])

    nc.compile()
    return nc


_NC_CACHE = None


def _get_nc():
    global _NC_CACHE
    if _NC_CACHE is None:
        _NC_CACHE = build_nc()
    return _NC_CACHE


def make_in_maps(x, W, b, global_A, global_B, local_A, local_B):
    x = np.ascontiguousarray(np.asarray(x, dtype=np.float32))
    W = np.ascontiguousarray(np.asarray(W, dtype=np.float32))
    b = np.asarray(b, dtype=np.float32)
    A_cat = np.ascontiguousarray(
        SCALE * np.concatenate([np.asarray(global_A), np.asarray(local_A)], axis=0)
    ).astype(np.float16)
    B_catT = np.ascontiguousarray(
        np.concatenate([np.asarray(global_B).T, np.asarray(local_B).T], axis=0)
    ).astype(np.float16)
    return [
        {"x": x[i], "W": W, "b16": b.astype(np.float16), "A_cat": A_cat,
         "B_catT": B_catT}
        for i in range(N_CORES)
    ]


def kernel(x, W, b, global_A, global_B, local_A, local_B):
    nc = _get_nc()
    in_maps = make_in_maps(x, W, b, global_A, global_B, local_A, local_B)
    res = run_bass_kernel_spmd(nc, in_maps, list(range(N_CORES))).results
    return np.stack([res[i]["out"] for i in range(N_CORES)], axis=0)


# revision 21
# speedup vs baseline: 1.4102x; 1.4102x over previous
"""LoRALinear kernel for Trainium2 (8 NeuronCores, SPMD data-parallel).

Computes out = x @ W.T + b + SCALE*((x@gA.T)@gB.T + (x@lA.T)@lB.T)
  x: [8, 2048, 1024] f32, W: [4096, 1024], b: [4096]
  gA/lA: [8, 1024], gB/lB: [4096, 8]  ->  out: [8, 2048, 4096] f32

Strategy (per core, one batch of x each):
  1. Merge LoRA into W_effT[k, o] = W.T + (SCALE*A_cat).T @ B_catT on
     device: W cast to fp16 on ScalarE, PE-transposed 4 blocks per
     [128,512] fp16 PSUM tile, rank-16 fp16 LoRA matmul evicted by
     ScalarE, summed into fp16 SBUF with one DVE op per tile.
  2. Main matmul per 128-row s-tile: cast x to fp16 (ScalarE),
     PE-transpose blocks to x.T (grouped 4-per-PSUM-tile, one DVE
     eviction each), accumulate psum[s,o] over 8 k-tiles of fp16
     matmuls (all-fp16 keeps LDWEIGHTS pipelined -> ~N cycles/matmul);
     bias added in f32 on DVE during psum eviction.

fp16 operand rounding gives ~3e-4 absmax relative error vs the f32
reference (validated numerically); accumulation stays f32 in PSUM.

Host only shards x over cores, stacks/pre-scales the rank-8 adapters
(A_cat = SCALE*[gA;lA], B_catT = [gB.T;lB.T]) and casts b to fp16 -
O(r*d) marshaling.
"""
import numpy as np
from contextlib import ExitStack

import concourse.bass as bass
import concourse.tile as tile
from concourse import bacc, mybir
from concourse.bass import ts, ds
from concourse.bass_utils import run_bass_kernel_spmd
from concourse.masks import make_identity

F32 = mybir.dt.float32
F16 = mybir.dt.float16

N_CORES = 8
B, S, DIN, DOUT, R = 8, 2048, 1024, 4096, 8
SCALE = 16.0 / 8
R2 = 2 * R

P = 128            # partition tile
OTILE = 512        # matmul moving free dim (one PSUM bank of f32)
KT = DIN // P      # 8 k-tiles
OT = DOUT // OTILE # 8 o-tiles
ST = S // P        # 16 s-tiles


def build_nc():
    nc = bacc.Bacc("TRN2", target_bir_lowering=False, debug=False,
                   num_devices=N_CORES)
    x = nc.dram_tensor("x", [S, DIN], F32, kind="ExternalInput").ap()
    W = nc.dram_tensor("W", [DOUT, DIN], F32, kind="ExternalInput").ap()
    bvec = nc.dram_tensor("b16", [DOUT], F16, kind="ExternalInput").ap()
    A_cat = nc.dram_tensor("A_cat", [R2, DIN], F16, kind="ExternalInput").ap()
    B_catT = nc.dram_tensor("B_catT", [R2, DOUT], F16, kind="ExternalInput").ap()
    out = nc.dram_tensor("out", [S, DOUT], F32, kind="ExternalOutput").ap()

    with tile.TileContext(nc) as tc:
        with ExitStack() as ctx:
            const = ctx.enter_context(tc.tile_pool(name="const", bufs=1))
            wet_pool = ctx.enter_context(tc.tile_pool(name="wet", bufs=1))
            wslab_pool = ctx.enter_context(tc.tile_pool(name="wslab", bufs=3))
            xin_pool = ctx.enter_context(tc.tile_pool(name="xin", bufs=2))
            xt_pool = ctx.enter_context(tc.tile_pool(name="xt", bufs=2))
            out_pool = ctx.enter_context(tc.tile_pool(name="outp", bufs=4))
            psh = ctx.enter_context(tc.tile_pool(name="psh", bufs=2, space="PSUM"))
            ps512 = ctx.enter_context(tc.tile_pool(name="ps512", bufs=6, space="PSUM"))

            # ---- constants ----
            ident_h = const.tile([P, P], F16)
            make_identity(nc, ident_h)

            acat = const.tile([R2, DIN], F16)
            nc.sync.dma_start(acat[:], A_cat)
            bcatt = const.tile([R2, DOUT], F16)
            nc.sync.dma_start(bcatt[:], B_catT)

            # bias broadcast to all 128 partitions via rank-1 fp16 matmul
            ones_col = const.tile([1, P], F16)
            nc.vector.memset(ones_col[:], 1.0)
            brow16 = const.tile([1, DOUT], F16)
            nc.sync.dma_start(brow16[:], bvec[None, :])
            bias_sb = const.tile([P, DOUT], F32)
            for ot in range(OT):
                pb = ps512.tile([P, OTILE], F32, tag="ps512")
                nc.tensor.matmul(pb[:], ones_col[:],
                                 brow16[:, ts(ot, OTILE)],
                                 start=True, stop=True)
                nc.vector.tensor_copy(bias_sb[:, ts(ot, OTILE)], pb[:])

            # ---- phase 1: W_effT[k, o] in SBUF, fp16 (8 tiles [128, DOUT]) ----
            wet = [wet_pool.tile([P, DOUT], F16, tag=f"wet{k}", name=f"wet{k}")
                   for k in range(KT)]
            for ot in range(OT):
                # 4 o-blocks of W cast to fp16 on ScalarE: [128, DIN] each
                wos = []
                for j in range(4):
                    wsl = wslab_pool.tile([P, DIN], F32, tag="wslab32")
                    nc.sync.dma_start(
                        wsl[:], W[ds(ot * OTILE + j * P, P), :])
                    w16 = wslab_pool.tile([P, DIN], F16, tag=f"wslab16_{j}",
                                          name=f"w16_{ot}_{j}")
                    nc.scalar.copy(w16[:], wsl[:])
                    wos.append(w16)
                for kt in range(KT):
                    # LoRA term: (SCALE*A_cat).T @ B_catT chunk, K=16
                    pl = ps512.tile([P, OTILE], F32, tag="ps512")
                    nc.tensor.matmul(pl[:], acat[:, ts(kt, P)],
                                     bcatt[:, ts(ot, OTILE)],
                                     start=True, stop=True)
                    nc.scalar.copy(wet[kt][:, ts(ot, OTILE)], pl[:])
                    # W.T: 4 fp16 PE transposes into one fp16 psum tile
                    pwq = psh.tile([P, OTILE], F16, tag="psh")
                    for j in range(4):
                        nc.tensor.matmul(pwq[:, ts(j, P)],
                                         wos[j][:, ts(kt, P)],
                                         ident_h[:], is_transpose=True,
                                         start=(j == 0), stop=(j == 3))
                    wchunk = wet[kt][:, ts(ot, OTILE)]
                    nc.vector.tensor_tensor(wchunk, pwq[:], wchunk,
                                            mybir.AluOpType.add)

            # ---- phase 2: out[s, o] = x @ W_effT + bias ----
            for st in range(ST):
                xin = xin_pool.tile([P, DIN], F32)
                nc.sync.dma_start(xin[:], x[ts(st, P), :])
                x16 = xin_pool.tile([P, DIN], F16, tag="x16")
                nc.scalar.copy(x16[:], xin[:])
                xt = xt_pool.tile([P, KT * P], F16)
                for g in range(KT // 4):
                    pxq = psh.tile([P, OTILE], F16, tag="psh")
                    for j in range(4):
                        nc.tensor.matmul(pxq[:, ts(j, P)],
                                         x16[:, ts(4 * g + j, P)], ident_h[:],
                                         is_transpose=True,
                                         start=(j == 0), stop=(j == 3))
                    nc.vector.tensor_copy(xt[:, ts(g, OTILE)], pxq[:])
                for ot in range(OT):
                    po = ps512.tile([P, OTILE], F32, tag="ps512")
                    for kt in range(KT):
                        nc.tensor.matmul(po[:], xt[:, ts(kt, P)],
                                         wet[kt][:, ts(ot, OTILE)],
                                         start=(kt == 0), stop=(kt == KT - 1))
                    osb = out_pool.tile([P, OTILE], F32)
                    nc.vector.tensor_tensor(osb[:], po[:],
                                            bias_sb[:, ts(ot, OTILE)],
                                            mybir.AluOpType.add)
                    nc.sync.dma_start(out[ts(st, P), ts(ot, OTILE)], osb[:])

    nc.compile()
    return nc


_NC_CACHE = None


def _get_nc():
    global _NC_CACHE
    if _NC_CACHE is None:
        _NC_CACHE = build_nc()
    return _NC_CACHE


def make_in_maps(x, W, b, global_A, global_B, local_A, local_B):
    x = np.ascontiguousarray(np.asarray(x, dtype=np.float32))
    W = np.ascontiguousarray(np.asarray(W, dtype=np.float32))
    b = np.asarray(b, dtype=np.float32)
    A_cat = np.ascontiguousarray(
        SCALE * np.concatenate([np.asarray(global_A), np.asarray(local_A)], axis=0)
    ).astype(np.float16)
    B_catT = np.ascontiguousarray(
        np.concatenate([np.asarray(global_B).T, np.asarray(local_B).T], axis=0)
    ).astype(np.float16)
    return [
        {"x": x[i], "W": W, "b16": b.astype(np.float16), "A_cat": A_cat,
         "B_catT": B_catT}
        for i in range(N_CORES)
    ]


def kernel(x, W, b, global_A, global_B, local_A, local_B):
    nc = _get_nc()
    in_maps = make_in_maps(x, W, b, global_A, global_B, local_A, local_B)
    res = run_bass_kernel_spmd(nc, in_maps, list(range(N_CORES))).results
    return np.stack([res[i]["out"] for i in range(N_CORES)], axis=0)
